# revision 10
# baseline (speedup 1.0000x reference)
"""Trainium2 Bass kernel for nn_MultiHeadAttention (N=2048, D=1024, H=16, causal).

Sharding: 16 heads split across 8 NeuronCores (2 heads/core, tensor-parallel
on the head dim).  Each core projects Q^T/K^T (its 128 head-dims x full
sequence) and V for its heads, computes causal attention in scores-transposed
layout ([nk, nq] blocks, softmax along the nk partition axis), applies its
128-row slice of Wo, and writes an fp16 partial [2048, 1024] output.  The
host sums the 8 partials and adds bo + bv@Wo_slice^T ("all-reduce after W_o"
done host-side; the V bias is mathematically a constant output row, so it
never touches the device).

v2 redesign (75.8us -> target ~48us modeled):
  - PV flipped: probs blocks [128k, 128q] are the *stationary* operand and
    the ones-augmented V [128k, 65] the moving one, so each PV matmul costs
    65 free-columns instead of 128, and the softmax denominator lands on the
    *partition* axis of the PV output.  Normalisation collapses to a
    per-partition reciprocal + scale (no denominator staging row, no PE
    broadcast, no [64,512] reciprocal).
  - The normalized attention output [q, dims] is flipped back to the
    Wo-stationary layout [dims, q] with the DMA XBAR transpose (14ns per
    16x128 tile), not PE/DVE work.
  - Scores computed per 1024-wide q-supertile (T0/T1) so each causally
    trimmed [128, width<=1024] score block takes ONE exp instruction
    (52 exps total vs 80): the ~185ns fixed Act cost per instruction was
    ~15us of the baseline's Act-engine 46us.  T0 is emitted in 512-wide
    halves so the exp train starts as soon as the first q/k column group
    lands (~2.5us).
  - Input DMA order: all q/k fp8 column tiles first (the score pipeline's
    critical path), v f16 tiles deferred behind them; V projection, PV,
    normalize, transpose and Wo ride the emission-interleaved filler queue.
  - Wo staging copies (PSUM f32 -> SBUF f16) are spread DVE/Pool by slack.
"""
import os
import sys

for _p in ("/opt/trn_rl_repo", "/root/.axon_site/_ro/trn_rl_repo"):
    if os.path.isdir(_p) and _p not in sys.path:
        sys.path.append(_p)

import numpy as np

import concourse.bass as bass
import concourse.mybir as mybir
from concourse import bacc
from concourse.bass_utils import run_bass_kernel_spmd
from concourse.tile import TileContext
from contextlib import ExitStack

N = 2048
D = 1024
NCORES = 8
DL = 128

F32 = mybir.dt.float32
F16 = mybir.dt.float16
F8 = mybir.dt.float8e4

# fp8 Q/K path: q, k, Wq, Wk in e4m3 (weights host-scaled x16 to clear the
# e4m3 denormal floor; bq, bk scaled to match; the extra 16*16 factor on the
# scores folds into the exp scale).  V path, probs, attnT, Wo stay f16 --
# every attempted fp8 extension of those paths measured >2e-2 end-to-end.
EXP_SCALE = 0.125 / 256.0


def build_nc(opts=None):
    nc = bacc.Bacc("TRN2", target_bir_lowering=False, debug=False,
                   num_devices=NCORES)

    qP = nc.dram_tensor("qP", [128, 8, N], F8, kind="ExternalInput")
    kP = nc.dram_tensor("kP", [128, 8, N], F8, kind="ExternalInput")
    vP = nc.dram_tensor("vP", [128, 8, N], F16, kind="ExternalInput")
    wqkP = nc.dram_tensor("wqkP", [128, 8, 2 * DL], F8, kind="ExternalInput")
    wvP = nc.dram_tensor("wvP", [128, 8, DL], F16, kind="ExternalInput")
    woP = nc.dram_tensor("woP", [DL, D], F16, kind="ExternalInput")
    bqk = nc.dram_tensor("bqk", [DL, 2], F32, kind="ExternalInput")
    out = nc.dram_tensor("out", [N, D], F16, kind="ExternalOutput")

    AF = mybir.ActivationFunctionType
    ALU = mybir.AluOpType
    DR = mybir.MatmulPerfMode.DoubleRow

    with TileContext(nc) as tc, ExitStack() as ctx:
        const = ctx.enter_context(tc.tile_pool(name="const", bufs=1))
        big = ctx.enter_context(tc.tile_pool(name="big", bufs=1))
        colp = ctx.enter_context(tc.tile_pool(name="colp", bufs=1))
        probsp = ctx.enter_context(tc.tile_pool(name="probsp", bufs=1))
        rcqp = ctx.enter_context(tc.tile_pool(name="rcqp", bufs=2))
        outp = ctx.enter_context(tc.tile_pool(name="outp", bufs=3))

        # ---- input streaming, one sync-queue in arrival-priority order:
        # ALL q/k fp8 tiles first (they feed the exp train, the kernel's
        # pacer; the very first q/k column group split in 256-col halves so
        # the first score lands ~5.5us in), then wv/wo, then the v f16
        # tiles (V-proj/PV/Wo ride the filler queue from ~20us on) ----
        wqk = const.tile([128, 8, 2 * DL], F8)
        nc.sync.dma_start(wqk[:], wqkP[:])
        wq = wqk[:, :, 0:DL]
        wk = wqk[:, :, DL:2 * DL]
        qc, kc, vc = [], [], []

        def load_col(lst, name, dram, c, dt, split=False):
            t = colp.tile([128, 8, 512], dt, name=f"{name}{c}")
            if split:
                nc.sync.dma_start(t[:, :, 0:256], dram[:, :, 512 * c:512 * c + 256])
            else:
                nc.sync.dma_start(t[:], dram[:, :, 512 * c:512 * (c + 1)])
            lst.append(t)

        load_col(qc, "qc", qP, 0, F8, split=True)
        load_col(kc, "kc", kP, 0, F8, split=True)
        bqk_t = const.tile([DL, 2], F32)
        nc.sync.dma_start(bqk_t[:], bqk[:])
        nc.sync.dma_start(qc[0][:, :, 256:512], qP[:, :, 256:512])
        nc.sync.dma_start(kc[0][:, :, 256:512], kP[:, :, 256:512])
        for c in range(1, 4):
            load_col(qc, "qc", qP, c, F8)
            load_col(kc, "kc", kP, c, F8)
        wv = const.tile([128, 8, DL], F16)
        nc.sync.dma_start(wv[:], wvP[:])
        wo = const.tile([DL, D], F16)
        nc.sync.dma_start(wo[:], woP[:])
        for c in range(4):
            load_col(vc, "vc", vP, c, F16)

        ones64 = const.tile([1, 64], F16)
        nc.vector.memset(ones64[:], 1.0)
        ones512 = const.tile([1, 512], F16)
        nc.vector.memset(ones512[:], 1.0)

        # ---- persistent activations ----
        QT = big.tile([128, N], F16)
        KT = big.tile([128, N], F16)
        # Vaug[p_seq, head, seq_block, 65]: cols 0:64 projected V, col 64
        # ones (PV's moving operand; the ones column accumulates the softmax
        # denominator into PV-output column 64 for free).
        Vaug = big.tile([128, 2, 16, 65], F16)
        nc.vector.memset(Vaug[:, :, :, 64:65], 1.0)
        attnT = big.tile([128, N], F16)
        attnQ = [big.tile([128, 8, 128], F16, name=f"attnQ{T}")
                 for T in range(2)]

        # probs segments: (T, h, b) -> list of (qlo, qhi, pb_tile)
        probs_seg = {}

        with tc.tile_pool(name="scp", bufs=2, space="PSUM") as scp, \
             tc.tile_pool(name="wps", bufs=1, space="PSUM") as wps, \
             tc.tile_pool(name="pvqp", bufs=2, space="PSUM") as pvqp:

            # The PE SEQ is in-order and sem-waits block it, so emission
            # interleaves independent "filler" work (V projection, PV
            # strip sweeps, normalize, transpose, Wo) between score blocks,
            # paced against the Activation-engine exp train.
            fillers = []

            def emit_score(T, h, b, qlo, qhi):
                # one causally-trimmed score block: k in [128b, 128b+128),
                # q in [qlo, qhi).  Matmuls split at PSUM bank boundaries;
                # ONE exp covers the whole [128, qhi-qlo] region.
                w = qhi - qlo
                r0, r1 = 64 * h, 64 * (h + 1)
                sc = scp.tile([128, 2, 512], F32, name="sc")
                flat = sc.rearrange("p a b -> p (a b)")
                for c0 in range(0, w, 512):
                    c1 = min(c0 + 512, w)
                    nc.tensor.matmul(
                        flat[:, c0:c1],
                        KT[r0:r1, 128 * b:128 * (b + 1)],
                        QT[r0:r1, qlo + c0:qlo + c1],
                        start=True, stop=True, tile_position=(64 * h, 0))
                pb = probsp.tile([128, w], F16, name=f"pb{T}{h}{b}_{qlo}")
                nc.scalar.activation(pb[:], flat[:, 0:w], AF.Exp,
                                     scale=EXP_SCALE)
                if 128 * b >= qlo:
                    # diagonal chunk starts this segment: causal-mask it
                    nc.gpsimd.affine_select(
                        out=pb[:, 0:128], in_=pb[:, 0:128],
                        compare_op=ALU.is_ge, fill=0.0,
                        base=0, pattern=[[1, 128]], channel_multiplier=-1)
                probs_seg.setdefault((T, h, b), []).append((qlo, qhi, pb))

            def probs_slice(T, h, b, s):
                q0 = 1024 * T + 128 * s
                for qlo, qhi, pb in probs_seg[(T, h, b)]:
                    if qlo <= q0 < qhi:
                        return pb[:, q0 - qlo:q0 - qlo + 128]
                raise AssertionError((T, h, b, s))

            def emit_vproj(c, j):
                # seq block blk = 4c+j -> vp region [:, 128j:128j+128]
                blk = 4 * c + j
                if j == 0:
                    emit_vproj.vp = scp.tile([128, 2, 512], F32, name="sc")
                vp = emit_vproj.vp.rearrange("p a b -> p (a b)")
                for u in range(8):
                    nc.tensor.matmul(
                        vp[:, 128 * j:128 * (j + 1)],
                        vc[c][:, u, 128 * j:128 * (j + 1)],
                        wv[:, u, :], start=(u == 0), stop=(u == 7))

            def emit_vcopy(c):
                vp = emit_vproj.vp.rearrange(
                    "p a (bb g d) -> p a g bb d", bb=4, g=2, d=64)
                nc.vector.tensor_scalar_mul(
                    Vaug[:, 0:2, 4 * c:4 * (c + 1), 0:64],
                    vp[:, 0, :, :, :], 1.0)

            pvq_ref = {}

            def emit_pv(T, h, s):
                # strip s of supertile T: accumulate over all k blocks
                # b = 0..8T+s into pvq[128q, 65] (probs stationary, Vaug
                # moving: 65 free-cols per matmul; col 64 = denominator).
                sg, si = s // 4, s % 4
                if si == 0:
                    pvq_ref[(T, h, sg)] = pvqp.tile([128, 4, 65], F32,
                                                    name="pvq")
                pvq = pvq_ref[(T, h, sg)]
                last = 8 * T + s
                for b in range(last + 1):
                    nc.tensor.matmul(
                        pvq[:, si, 0:65],
                        probs_slice(T, h, b, s),
                        Vaug[:, h, b, 0:65],
                        start=(b == 0), stop=(b == last))

            def emit_norm(T, h, sg):
                pvq = pvq_ref[(T, h, sg)]
                rcq = rcqp.tile([128, 4, 1], F32, name="rcq")
                nc.vector.reciprocal(rcq[:], pvq[:, :, 64:65])
                nc.vector.tensor_mul(
                    attnQ[T][:, 4 * sg:4 * (sg + 1), 64 * h:64 * (h + 1)],
                    pvq[:, :, 0:64],
                    rcq.broadcast_to((128, 4, 64)))

            def emit_norm_strip(T, h, s):
                # per-strip normalize for the final strip-group, so the
                # tail pipelines strip-by-strip instead of per-quad
                sg, si = s // 4, s % 4
                pvq = pvq_ref[(T, h, sg)]
                rcq = rcqp.tile([128, 4, 1], F32, name="rcq")
                nc.vector.reciprocal(rcq[:, 0:1, :], pvq[:, si:si + 1, 64:65])
                nc.vector.tensor_mul(
                    attnQ[T][:, s:s + 1, 64 * h:64 * (h + 1)],
                    pvq[:, si:si + 1, 0:64],
                    rcq[:, 0:1, :].broadcast_to((128, 1, 64)))

            def emit_transpose(T, s):
                m = 8 * T + s
                nc.sync.dma_start_transpose(
                    attnT[:, 128 * m:128 * (m + 1)], attnQ[T][:, s, :])

            def emit_wo(T, s, stage0, stage1):
                # two [128,512] halves through rotating single-bank PSUM
                # slots, each staged f32->f16 on its own engine, one merged
                # output DMA per 128-row band
                m = 8 * T + s
                ob = outp.tile([128, 1024], F16, name="ob")
                for u, stage in ((0, stage0), (1, stage1)):
                    wp = wps.tile([128, 512], F32, name="wp")
                    nc.tensor.matmul(wp[:],
                                     attnT[:, 128 * m:128 * (m + 1)],
                                     wo[:, 512 * u:512 * (u + 1)],
                                     start=True, stop=True)
                    dst = ob[:, 512 * u:512 * (u + 1)]
                    if stage == "act":
                        nc.scalar.activation(dst, wp[:], AF.Copy)
                    elif stage == "pool":
                        nc.gpsimd.tensor_scalar_mul(dst, wp[:], 1.0)
                    else:
                        nc.vector.tensor_scalar_mul(dst, wp[:], 1.0)
                nc.sync.dma_start(out[128 * m:128 * (m + 1), :], ob[:])

            proj_ps = {}

            def emit_proj(g, lo=0, hi=512):
                # QT/KT columns [512g+lo : 512g+hi] from qc[g]/kc[g], fp8
                # DoubleRow (256-wide contraction per pass); group 0 comes
                # in 256-col halves chasing the front DMA
                if lo == 0:
                    proj_ps[g] = scp.tile([128, 2, 512], F32, name="sc")
                ps = proj_ps[g]
                for src_c, w, half in ((qc[g], wq, 0), (kc[g], wk, 1)):
                    for jj in range(4):
                        nc.tensor.matmul(
                            ps[:, half, lo:hi], w[:, 2 * jj:2 * jj + 2, :],
                            src_c[:, 2 * jj:2 * jj + 2, lo:hi],
                            start=(jj == 0), stop=(jj == 3),
                            perf_mode=DR)
                nc.vector.tensor_scalar_add(
                    QT[:, 512 * g + lo:512 * g + hi], ps[:, 0, lo:hi],
                    bqk_t[:, 0:1])
                nc.vector.tensor_scalar_add(
                    KT[:, 512 * g + lo:512 * g + hi], ps[:, 1, lo:hi],
                    bqk_t[:, 1:2])

            def warmup(n=1):
                # keep the tensor engine busy through the DMA front so the
                # p-state ramp completes before the first projection
                for _ in range(n):
                    wu = wps.tile([128, 512], F32, name="wp")
                    nc.tensor.matmul(wu[0:64, :], ones64[:], ones512[:],
                                     start=True, stop=True)

            import math

            blocks_left = [32]

            def pop_fillers():
                k = max(2, min(5, math.ceil(
                    len(fillers) / max(1, blocks_left[0]))))
                for _ in range(k):
                    if fillers:
                        fillers.pop(0)()

            def queue_vproj(c):
                fillers.extend(lambda c=c, j=j: emit_vproj(c, j)
                               for j in range(4))
                fillers.append(lambda c=c: emit_vcopy(c))

            def queue_pv(T, h, sg):
                for si in range(4):
                    fillers.append(lambda T=T, h=h, s=4 * sg + si:
                                   emit_pv(T, h, s))
                fillers.append(lambda T=T, h=h, sg=sg:
                               emit_norm(T, h, sg))

            # transposes+Wo for strip-group sg of supertile T (after the
            # second head's norm); staging engines: DVE/Pool mid-flight,
            # Act only on the final strips (idle once the exp train drains)
            def queue_tail(T, sg):
                for si in range(4):
                    fillers.append(lambda T=T, s=4 * sg + si:
                                   emit_transpose(T, s))
                for si in range(4):
                    s = 4 * sg + si
                    st = ("dve", "pool") if si % 2 == 0 else ("pool", "dve")
                    fillers.append(lambda T=T, s=s, st=st:
                                   emit_wo(T, s, st[0], st[1]))

            # ---------------- emission schedule ----------------
            # Heads are interleaved within every score row so both heads'
            # probs for a k-block land together: PV/norm/transpose/Wo for a
            # strip-group unblocks mid-row instead of after the second
            # head's full row (which made the old tail ~20us of serial Wo).
            warmup(4)
            emit_proj(0, 0, 256)
            warmup(1)

            # T0 in 256/256/512-wide column passes chasing the input DMA
            for h in range(2):
                for b in range(2):
                    emit_score(0, h, b, 128 * b, 256)
            emit_proj(0, 256, 512)
            for b in range(4):
                for h in range(2):
                    emit_score(0, h, b, max(256, 128 * b), 512)
                if b == 0:
                    emit_proj(1)
            for b in range(8):
                for h in range(2):
                    emit_score(0, h, b, max(512, 128 * b), 1024)

            emit_proj(2)
            emit_proj(3)
            for b in range(16):
                for h in range(2):
                    emit_score(1, h, b, max(1024, 128 * b), 2048)
                blocks_left[0] -= 1
                pop_fillers()
                # queue points tuned to the v-tile DMA arrivals (vc[c]
                # lands at ~20/23/26/29us; PE pops run ~2us ahead of the
                # exp train, which passes k-block b at ~18.5+2.1*b us)
                if b == 1:
                    queue_vproj(0)
                    queue_pv(0, 0, 0)
                    queue_pv(0, 1, 0)
                    queue_tail(0, 0)
                if b == 3:
                    queue_vproj(1)
                    queue_pv(0, 0, 1)
                    queue_pv(0, 1, 1)
                    queue_tail(0, 1)
                if b == 5:
                    queue_vproj(2)
                if b == 7:
                    queue_vproj(3)
                if b == 11:
                    # pb(T1, *, b<=11) emitted: strip-group 0 (strips 0..3
                    # sweep k-blocks 0..11) ready for both heads
                    queue_pv(1, 0, 0)
                    queue_pv(1, 1, 0)
                    queue_tail(1, 0)
            # final strip-group (T1, sg1): h0 as a quad, then per-strip
            # pipelining for h1 so the post-exp tail is one strip deep
            queue_pv(1, 0, 1)
            for si in range(4):
                s = 4 + si
                fillers.append(lambda s=s: emit_pv(1, 1, s))
                fillers.append(lambda s=s: emit_norm_strip(1, 1, s))
                fillers.append(lambda s=s: emit_transpose(1, s))
                st = (("pool", "dve"), ("dve", "pool"),
                      ("act", "pool"), ("act", "dve"))[si]
                fillers.append(lambda s=s, st=st:
                               emit_wo(1, s, st[0], st[1]))
            while fillers:
                fillers.pop(0)()

    nc.compile()
    return nc


def make_in_maps(q, k, v, Wq, bq, Wk, bk, Wv, bv, Wo, bo):
    import ml_dtypes
    fp8 = ml_dtypes.float8_e4m3
    f32 = np.float32
    WSCALE = 16.0

    def pack_cols(x, dt):
        # [N, D] input -> x.T [D, N] -> [128, 8, N] with row (j*128+p) at
        # [p, j, :]
        xt = np.ascontiguousarray(x.T.astype(f32))
        return np.ascontiguousarray(
            xt.reshape(8, 128, N).transpose(1, 0, 2)).astype(dt)

    qPa, kPa = pack_cols(q, fp8), pack_cols(k, fp8)
    vPa = pack_cols(v, np.float16)
    WqT = Wq.T.astype(f32) * WSCALE
    WkT = Wk.T.astype(f32) * WSCALE
    WvT = Wv.T.astype(f32)
    WoT = Wo.T.astype(f32)

    def pack_w(WT, c, dt):
        # [D, DL] column slice -> [128, 8, DL]
        sl = np.ascontiguousarray(WT[:, DL * c:DL * (c + 1)])
        return np.ascontiguousarray(
            sl.reshape(8, 128, DL).transpose(1, 0, 2)).astype(dt)

    in_maps = []
    for c in range(NCORES):
        d0 = DL * c
        in_maps.append({
            "qP": qPa, "kP": kPa, "vP": vPa,
            "wqkP": np.ascontiguousarray(np.concatenate(
                [pack_w(WqT, c, fp8), pack_w(WkT, c, fp8)], axis=2)),
            "wvP": pack_w(WvT, c, np.float16),
            "woP": np.ascontiguousarray(WoT[d0:d0 + DL, :]).astype(np.float16),
            "bqk": np.ascontiguousarray(
                np.stack([bq[d0:d0 + DL] * WSCALE,
                          bk[d0:d0 + DL] * WSCALE], axis=1)).astype(f32),
        })
    return in_maps


_NC_CACHE = None


def _get_nc():
    global _NC_CACHE
    if _NC_CACHE is None:
        _NC_CACHE = build_nc()
    return _NC_CACHE


def kernel(q, k, v, Wq, bq, Wk, bk, Wv, bv, Wo, bo):
    """Full-input / full-output entry point (harness contract)."""
    q, k, v = np.asarray(q), np.asarray(k), np.asarray(v)
    Wq, bq, Wk, bk = np.asarray(Wq), np.asarray(bq), np.asarray(Wk), np.asarray(bk)
    Wv, bv, Wo, bo = np.asarray(Wv), np.asarray(bv), np.asarray(Wo), np.asarray(bo)
    nc = _get_nc()
    in_maps = make_in_maps(q, k, v, Wq, bq, Wk, bk, Wv, bv, Wo, bo)
    res = run_bass_kernel_spmd(nc, in_maps, list(range(NCORES)))
    acc = res.results[0]["out"].astype(np.float64)
    for c in range(1, NCORES):
        acc += res.results[c]["out"]
    # V bias folded host-side: concat rows carry +bv per head-dim, so the
    # device-side output is short exactly bv @ Wo^T (a constant row).
    acc += (bv.astype(np.float64) @ Wo.T.astype(np.float64))
    acc += bo.astype(np.float64)
    return acc.astype(np.float32)


# revision 14
# speedup vs baseline: 1.1447x; 1.1447x over previous
"""Trainium2 Bass kernel for nn_MultiHeadAttention (N=2048, D=1024, H=16, causal).

Sharding: 16 heads split across 8 NeuronCores (2 heads/core, tensor-parallel
on the head dim).  Each core projects Q^T/K^T (its 128 head-dims x full
sequence) and V for its heads, computes causal attention in scores-transposed
layout ([nk, nq] blocks, softmax along the nk partition axis), applies its
128-row slice of Wo, and writes an fp16 partial [2048, 1024] output.  The
host sums the 8 partials and adds bo + bv@Wo_slice^T ("all-reduce after W_o"
done host-side; the V bias is mathematically a constant output row, so it
never touches the device).

v2 redesign (75.8us -> target ~48us modeled):
  - PV flipped: probs blocks [128k, 128q] are the *stationary* operand and
    the ones-augmented V [128k, 65] the moving one, so each PV matmul costs
    65 free-columns instead of 128, and the softmax denominator lands on the
    *partition* axis of the PV output.  Normalisation collapses to a
    per-partition reciprocal + scale (no denominator staging row, no PE
    broadcast, no [64,512] reciprocal).
  - The normalized attention output [q, dims] is flipped back to the
    Wo-stationary layout [dims, q] with the DMA XBAR transpose (14ns per
    16x128 tile), not PE/DVE work.
  - Scores computed per 1024-wide q-supertile (T0/T1) so each causally
    trimmed [128, width<=1024] score block takes ONE exp instruction
    (52 exps total vs 80): the ~185ns fixed Act cost per instruction was
    ~15us of the baseline's Act-engine 46us.  T0 is emitted in 512-wide
    halves so the exp train starts as soon as the first q/k column group
    lands (~2.5us).
  - Input DMA order: all q/k fp8 column tiles first (the score pipeline's
    critical path), v f16 tiles deferred behind them; V projection, PV,
    normalize, transpose and Wo ride the emission-interleaved filler queue.
  - Wo staging copies (PSUM f32 -> SBUF f16) are spread DVE/Pool by slack.
"""
import os
import sys

for _p in ("/opt/trn_rl_repo", "/root/.axon_site/_ro/trn_rl_repo"):
    if os.path.isdir(_p) and _p not in sys.path:
        sys.path.append(_p)

import numpy as np

import concourse.bass as bass
import concourse.mybir as mybir
from concourse import bacc
from concourse.bass_utils import run_bass_kernel_spmd
from concourse.tile import TileContext
from contextlib import ExitStack

N = 2048
D = 1024
NCORES = 8
DL = 128

F32 = mybir.dt.float32
F16 = mybir.dt.float16
F8 = mybir.dt.float8e4

# fp8 Q/K path: q, k, Wq, Wk in e4m3 (weights host-scaled x16 to clear the
# e4m3 denormal floor; bq, bk scaled to match; the extra 16*16 factor on the
# scores folds into the exp scale).  V path, probs, attnT, Wo stay f16 --
# every attempted fp8 extension of those paths measured >2e-2 end-to-end.
EXP_SCALE = 0.125 / 256.0


def build_nc(opts=None):
    nc = bacc.Bacc("TRN2", target_bir_lowering=False, debug=False,
                   num_devices=NCORES)

    qP = nc.dram_tensor("qP", [128, 8, N], F8, kind="ExternalInput")
    kP = nc.dram_tensor("kP", [128, 8, N], F8, kind="ExternalInput")
    vP = nc.dram_tensor("vP", [128, 8, N], F16, kind="ExternalInput")
    wqkP = nc.dram_tensor("wqkP", [128, 8, 2 * DL], F8, kind="ExternalInput")
    wvP = nc.dram_tensor("wvP", [128, 8, DL], F16, kind="ExternalInput")
    woP = nc.dram_tensor("woP", [DL, D], F16, kind="ExternalInput")
    bqk = nc.dram_tensor("bqk", [DL, 2], F32, kind="ExternalInput")
    out = nc.dram_tensor("out", [N, D], F16, kind="ExternalOutput")

    AF = mybir.ActivationFunctionType
    ALU = mybir.AluOpType
    DR = mybir.MatmulPerfMode.DoubleRow

    with TileContext(nc) as tc, ExitStack() as ctx:
        const = ctx.enter_context(tc.tile_pool(name="const", bufs=1))
        big = ctx.enter_context(tc.tile_pool(name="big", bufs=1))
        colp = ctx.enter_context(tc.tile_pool(name="colp", bufs=1))
        probsp = ctx.enter_context(tc.tile_pool(name="probsp", bufs=1))
        rcqp = ctx.enter_context(tc.tile_pool(name="rcqp", bufs=2))
        outp = ctx.enter_context(tc.tile_pool(name="outp", bufs=3))

        # ---- input streaming, one sync-queue in arrival-priority order:
        # ALL q/k fp8 tiles first (they feed the exp train, the kernel's
        # pacer; the very first q/k column group split in 256-col halves so
        # the first score lands ~5.5us in), then wv/wo, then the v f16
        # tiles (V-proj/PV/Wo ride the filler queue from ~20us on) ----
        wqk = const.tile([128, 8, 2 * DL], F8)
        nc.sync.dma_start(wqk[:], wqkP[:])
        wq = wqk[:, :, 0:DL]
        wk = wqk[:, :, DL:2 * DL]
        qc, kc, vc = [], [], []

        def load_col(lst, name, dram, c, dt, split=False):
            t = colp.tile([128, 8, 512], dt, name=f"{name}{c}")
            if split:
                nc.sync.dma_start(t[:, :, 0:256], dram[:, :, 512 * c:512 * c + 256])
            else:
                nc.sync.dma_start(t[:], dram[:, :, 512 * c:512 * (c + 1)])
            lst.append(t)

        load_col(qc, "qc", qP, 0, F8, split=True)
        load_col(kc, "kc", kP, 0, F8, split=True)
        bqk_t = const.tile([DL, 2], F32)
        nc.sync.dma_start(bqk_t[:], bqk[:])
        nc.sync.dma_start(qc[0][:, :, 256:512], qP[:, :, 256:512])
        nc.sync.dma_start(kc[0][:, :, 256:512], kP[:, :, 256:512])
        for c in range(1, 4):
            load_col(qc, "qc", qP, c, F8)
            load_col(kc, "kc", kP, c, F8)
        wv = const.tile([128, 8, DL], F16)
        nc.sync.dma_start(wv[:], wvP[:])
        wo = const.tile([DL, D], F16)
        nc.sync.dma_start(wo[:], woP[:])
        for c in range(4):
            load_col(vc, "vc", vP, c, F16)

        ones64 = const.tile([1, 64], F16)
        nc.vector.memset(ones64[:], 1.0)
        ones512 = const.tile([1, 512], F16)
        nc.vector.memset(ones512[:], 1.0)
        # f16 identity for PE transposes: ones masked to the diagonal
        ident = const.tile([128, 128], F16)
        nc.vector.memset(ident[:], 1.0)
        nc.gpsimd.affine_select(
            out=ident[:], in_=ident[:], compare_op=mybir.AluOpType.is_ge,
            fill=0.0, base=0, pattern=[[1, 128]], channel_multiplier=-1)
        nc.gpsimd.affine_select(
            out=ident[:], in_=ident[:], compare_op=mybir.AluOpType.is_le,
            fill=0.0, base=0, pattern=[[1, 128]], channel_multiplier=-1)
        # dummy exp so the Act table load happens during the DMA front,
        # not in front of the first real score exp
        scratch = const.tile([1, 64], F16)
        nc.scalar.activation(scratch[:], ones64[:],
                             mybir.ActivationFunctionType.Exp, scale=1.0)

        # ---- persistent activations ----
        QT = big.tile([128, N], F16)
        KT = big.tile([128, N], F16)
        # Vaug[p_seq, head, seq_block, 65]: cols 0:64 projected V, col 64
        # ones (PV's moving operand; the ones column accumulates the softmax
        # denominator into PV-output column 64 for free).
        Vaug = big.tile([128, 2, 16, 65], F16)
        nc.vector.memset(Vaug[:, :, :, 64:65], 1.0)
        attnT = big.tile([128, N], F16)
        attnQ = [big.tile([128, 8, 128], F16, name=f"attnQ{T}")
                 for T in range(2)]

        # probs segments: (T, h, b) -> list of (qlo, qhi, pb_tile)
        probs_seg = {}

        with tc.tile_pool(name="scp", bufs=2, space="PSUM") as scp, \
             tc.tile_pool(name="wps", bufs=1, space="PSUM") as wps, \
             tc.tile_pool(name="pvqp", bufs=2, space="PSUM") as pvqp:

            # The PE SEQ is in-order and sem-waits block it, so emission
            # interleaves independent "filler" work (V projection, PV
            # strip sweeps, normalize, transpose, Wo) between score blocks,
            # paced against the Activation-engine exp train.
            fillers = []

            def emit_score(T, h, b, qlo, qhi):
                # one causally-trimmed score block: k in [128b, 128b+128),
                # q in [qlo, qhi).  Matmuls split at PSUM bank boundaries;
                # ONE exp covers the whole [128, qhi-qlo] region.
                w = qhi - qlo
                r0, r1 = 64 * h, 64 * (h + 1)
                sc = scp.tile([128, 2, 512], F32, name="sc")
                flat = sc.rearrange("p a b -> p (a b)")
                for c0 in range(0, w, 512):
                    c1 = min(c0 + 512, w)
                    nc.tensor.matmul(
                        flat[:, c0:c1],
                        KT[r0:r1, 128 * b:128 * (b + 1)],
                        QT[r0:r1, qlo + c0:qlo + c1],
                        start=True, stop=True, tile_position=(64 * h, 0))
                pb = probsp.tile([128, w], F16, name=f"pb{T}{h}{b}_{qlo}")
                nc.scalar.activation(pb[:], flat[:, 0:w], AF.Exp,
                                     scale=EXP_SCALE)
                if 128 * b >= qlo:
                    # diagonal chunk starts this segment: causal-mask it
                    nc.gpsimd.affine_select(
                        out=pb[:, 0:128], in_=pb[:, 0:128],
                        compare_op=ALU.is_ge, fill=0.0,
                        base=0, pattern=[[1, 128]], channel_multiplier=-1)
                probs_seg.setdefault((T, h, b), []).append((qlo, qhi, pb))

            def probs_slice(T, h, b, s):
                q0 = 1024 * T + 128 * s
                for qlo, qhi, pb in probs_seg[(T, h, b)]:
                    if qlo <= q0 < qhi:
                        return pb[:, q0 - qlo:q0 - qlo + 128]
                raise AssertionError((T, h, b, s))

            def emit_vproj(c, j):
                # seq block blk = 4c+j -> vp region [:, 128j:128j+128]
                blk = 4 * c + j
                if j == 0:
                    emit_vproj.vp = scp.tile([128, 2, 512], F32, name="sc")
                vp = emit_vproj.vp.rearrange("p a b -> p (a b)")
                for u in range(8):
                    nc.tensor.matmul(
                        vp[:, 128 * j:128 * (j + 1)],
                        vc[c][:, u, 128 * j:128 * (j + 1)],
                        wv[:, u, :], start=(u == 0), stop=(u == 7))

            def emit_vcopy(c):
                vp = emit_vproj.vp.rearrange(
                    "p a (bb g d) -> p a g bb d", bb=4, g=2, d=64)
                nc.vector.tensor_scalar_mul(
                    Vaug[:, 0:2, 4 * c:4 * (c + 1), 0:64],
                    vp[:, 0, :, :, :], 1.0)

            pvq_ref = {}

            def emit_pv(T, h, s):
                # strip s of supertile T: accumulate over all k blocks
                # b = 0..8T+s into pvq[128q, 65] (probs stationary, Vaug
                # moving: 65 free-cols per matmul; col 64 = denominator).
                sg, si = s // 4, s % 4
                if si == 0:
                    pvq_ref[(T, h, sg)] = pvqp.tile([128, 4, 65], F32,
                                                    name="pvq")
                pvq = pvq_ref[(T, h, sg)]
                last = 8 * T + s
                for b in range(last + 1):
                    nc.tensor.matmul(
                        pvq[:, si, 0:65],
                        probs_slice(T, h, b, s),
                        Vaug[:, h, b, 0:65],
                        start=(b == 0), stop=(b == last))

            def emit_norm(T, h, sg):
                pvq = pvq_ref[(T, h, sg)]
                rcq = rcqp.tile([128, 4, 1], F32, name="rcq")
                nc.vector.reciprocal(rcq[:], pvq[:, :, 64:65])
                nc.vector.tensor_mul(
                    attnQ[T][:, 4 * sg:4 * (sg + 1), 64 * h:64 * (h + 1)],
                    pvq[:, :, 0:64],
                    rcq.broadcast_to((128, 4, 64)))

            def emit_norm_strip(T, h, s):
                # per-strip normalize for the final strip-group, so the
                # tail pipelines strip-by-strip instead of per-quad
                sg, si = s // 4, s % 4
                pvq = pvq_ref[(T, h, sg)]
                rcq = rcqp.tile([128, 4, 1], F32, name="rcq")
                nc.vector.reciprocal(rcq[:, 0:1, :], pvq[:, si:si + 1, 64:65])
                nc.vector.tensor_mul(
                    attnQ[T][:, s:s + 1, 64 * h:64 * (h + 1)],
                    pvq[:, si:si + 1, 0:64],
                    rcq[:, 0:1, :].broadcast_to((128, 1, 64)))

            def emit_transpose(T, s):
                # PE transpose (53ns) + DVE copy: much shorter latency than
                # the DMA XBAR path and keeps SP/HWDGE free for output DMAs
                m = 8 * T + s
                tp = wps.tile([128, 256], F16, name="wp")
                nc.tensor.transpose(tp[:, 0:128], attnQ[T][:, s, :],
                                    ident[:])
                nc.vector.tensor_scalar_mul(
                    attnT[:, 128 * m:128 * (m + 1)], tp[:, 0:128], 1.0)

            def emit_wo(T, s, stage0, stage1):
                # two [128,512] halves through rotating single-bank PSUM
                # slots, each staged f32->f16 on its own engine, one merged
                # output DMA per 128-row band
                m = 8 * T + s
                ob = outp.tile([128, 1024], F16, name="ob")
                for u, stage in ((0, stage0), (1, stage1)):
                    wp = wps.tile([128, 512], F32, name="wp")
                    nc.tensor.matmul(wp[:],
                                     attnT[:, 128 * m:128 * (m + 1)],
                                     wo[:, 512 * u:512 * (u + 1)],
                                     start=True, stop=True)
                    dst = ob[:, 512 * u:512 * (u + 1)]
                    if stage == "act":
                        nc.scalar.activation(dst, wp[:], AF.Copy)
                    elif stage == "pool":
                        nc.gpsimd.tensor_scalar_mul(dst, wp[:], 1.0)
                    else:
                        nc.vector.tensor_scalar_mul(dst, wp[:], 1.0)
                nc.sync.dma_start(out[128 * m:128 * (m + 1), :], ob[:])

            proj_ps = {}

            def emit_proj(g, lo=0, hi=512):
                # QT/KT columns [512g+lo : 512g+hi] from qc[g]/kc[g], fp8
                # DoubleRow (256-wide contraction per pass); group 0 comes
                # in 256-col halves chasing the front DMA
                if lo == 0:
                    proj_ps[g] = scp.tile([128, 2, 512], F32, name="sc")
                ps = proj_ps[g]
                for src_c, w, half in ((qc[g], wq, 0), (kc[g], wk, 1)):
                    for jj in range(4):
                        nc.tensor.matmul(
                            ps[:, half, lo:hi], w[:, 2 * jj:2 * jj + 2, :],
                            src_c[:, 2 * jj:2 * jj + 2, lo:hi],
                            start=(jj == 0), stop=(jj == 3),
                            perf_mode=DR)
                nc.vector.tensor_scalar_add(
                    QT[:, 512 * g + lo:512 * g + hi], ps[:, 0, lo:hi],
                    bqk_t[:, 0:1])
                nc.vector.tensor_scalar_add(
                    KT[:, 512 * g + lo:512 * g + hi], ps[:, 1, lo:hi],
                    bqk_t[:, 1:2])

            def warmup(n=1):
                # keep the tensor engine busy through the DMA front so the
                # p-state ramp completes before the first projection
                for _ in range(n):
                    wu = wps.tile([128, 512], F32, name="wp")
                    nc.tensor.matmul(wu[0:64, :], ones64[:], ones512[:],
                                     start=True, stop=True)

            import math

            pairs_left = [32]

            def pop_fillers():
                k = max(2, min(6, math.ceil(
                    len(fillers) / max(1, pairs_left[0]))))
                for _ in range(k):
                    if fillers:
                        fillers.pop(0)()

            def queue_vproj(c):
                fillers.extend(lambda c=c, j=j: emit_vproj(c, j)
                               for j in range(4))
                fillers.append(lambda c=c: emit_vcopy(c))

            def queue_pv(T, h, sg):
                for si in range(4):
                    fillers.append(lambda T=T, h=h, s=4 * sg + si:
                                   emit_pv(T, h, s))
                fillers.append(lambda T=T, h=h, sg=sg:
                               emit_norm(T, h, sg))

            # transpose+Wo for strip-group sg of T (needs both heads'
            # norms); wo staging on Pool mid-flight (keeps the DVE queue as
            # a pure fast-path for copies/norms that gate the PE stream)
            def queue_tail(T, sg):
                for si in range(4):
                    s = 4 * sg + si
                    fillers.append(lambda T=T, s=s: emit_transpose(T, s))
                    fillers.append(lambda T=T, s=s:
                                   emit_wo(T, s, "pool", "pool"))

            # ---------------- emission schedule ----------------
            # Heads are interleaved within every score row so both heads'
            # probs for a k-block land together and the per-strip-group
            # backend (PV/norm/transpose/Wo) unblocks mid-row.  Fillers pop
            # per (block, head) score pair; a popped filler executes ~2
            # score pairs behind the exp frontier (PSUM slot rotation), so
            # queue points only need their deps EMITTED, with v-tile DMA
            # arrival (~20-29us) comfortably ahead of execution time.
            warmup(4)
            emit_proj(0, 0, 256)
            warmup(1)

            # T0 in 256/256/512-wide column passes chasing the input DMA
            for h in range(2):
                for b in range(2):
                    emit_score(0, h, b, 128 * b, 256)
            emit_proj(0, 256, 512)
            for b in range(4):
                for h in range(2):
                    emit_score(0, h, b, max(256, 128 * b), 512)
                if b == 0:
                    emit_proj(1)
            for b in range(8):
                for h in range(2):
                    emit_score(0, h, b, max(512, 128 * b), 1024)
            # T0's backend: all pb deps emitted; Vaug deps (vc0/vc1) land
            # well before any of this executes
            queue_vproj(0)
            queue_pv(0, 0, 0)
            queue_pv(0, 1, 0)
            queue_tail(0, 0)
            queue_vproj(1)
            queue_pv(0, 0, 1)
            queue_pv(0, 1, 1)
            queue_tail(0, 1)
            queue_vproj(2)
            queue_vproj(3)

            emit_proj(2)
            emit_proj(3)
            for b in range(16):
                for h in range(2):
                    emit_score(1, h, b, max(1024, 128 * b), 2048)
                    pairs_left[0] -= 1
                    pop_fillers()
                if b == 11:
                    # pb(T1, *, b<=11) emitted: strip-group 0 (strips 0..3
                    # sweep k-blocks 0..11) ready for both heads
                    queue_pv(1, 0, 0)
                    queue_pv(1, 1, 0)
                    queue_tail(1, 0)
            # final strip-group (T1, sg1): h0 as a quad, then per-strip
            # pipelining for h1 so the post-exp tail is one strip deep;
            # staging moves to DVE/Act, both idle once the exp train ends
            queue_pv(1, 0, 1)
            for si in range(4):
                s = 4 + si
                fillers.append(lambda s=s: emit_pv(1, 1, s))
                fillers.append(lambda s=s: emit_norm_strip(1, 1, s))
                fillers.append(lambda s=s: emit_transpose(1, s))
                st = (("pool", "dve"), ("dve", "pool"),
                      ("act", "dve"), ("act", "dve"))[si]
                fillers.append(lambda s=s, st=st:
                               emit_wo(1, s, st[0], st[1]))
            while fillers:
                fillers.pop(0)()

    nc.compile()
    return nc


def make_in_maps(q, k, v, Wq, bq, Wk, bk, Wv, bv, Wo, bo):
    import ml_dtypes
    fp8 = ml_dtypes.float8_e4m3
    f32 = np.float32
    WSCALE = 16.0

    def pack_cols(x, dt):
        # [N, D] input -> x.T [D, N] -> [128, 8, N] with row (j*128+p) at
        # [p, j, :]
        xt = np.ascontiguousarray(x.T.astype(f32))
        return np.ascontiguousarray(
            xt.reshape(8, 128, N).transpose(1, 0, 2)).astype(dt)

    qPa, kPa = pack_cols(q, fp8), pack_cols(k, fp8)
    vPa = pack_cols(v, np.float16)
    WqT = Wq.T.astype(f32) * WSCALE
    WkT = Wk.T.astype(f32) * WSCALE
    WvT = Wv.T.astype(f32)
    WoT = Wo.T.astype(f32)

    def pack_w(WT, c, dt):
        # [D, DL] column slice -> [128, 8, DL]
        sl = np.ascontiguousarray(WT[:, DL * c:DL * (c + 1)])
        return np.ascontiguousarray(
            sl.reshape(8, 128, DL).transpose(1, 0, 2)).astype(dt)

    in_maps = []
    for c in range(NCORES):
        d0 = DL * c
        in_maps.append({
            "qP": qPa, "kP": kPa, "vP": vPa,
            "wqkP": np.ascontiguousarray(np.concatenate(
                [pack_w(WqT, c, fp8), pack_w(WkT, c, fp8)], axis=2)),
            "wvP": pack_w(WvT, c, np.float16),
            "woP": np.ascontiguousarray(WoT[d0:d0 + DL, :]).astype(np.float16),
            "bqk": np.ascontiguousarray(
                np.stack([bq[d0:d0 + DL] * WSCALE,
                          bk[d0:d0 + DL] * WSCALE], axis=1)).astype(f32),
        })
    return in_maps


_NC_CACHE = None


def _get_nc():
    global _NC_CACHE
    if _NC_CACHE is None:
        _NC_CACHE = build_nc()
    return _NC_CACHE


def kernel(q, k, v, Wq, bq, Wk, bk, Wv, bv, Wo, bo):
    """Full-input / full-output entry point (harness contract)."""
    q, k, v = np.asarray(q), np.asarray(k), np.asarray(v)
    Wq, bq, Wk, bk = np.asarray(Wq), np.asarray(bq), np.asarray(Wk), np.asarray(bk)
    Wv, bv, Wo, bo = np.asarray(Wv), np.asarray(bv), np.asarray(Wo), np.asarray(bo)
    nc = _get_nc()
    in_maps = make_in_maps(q, k, v, Wq, bq, Wk, bk, Wv, bv, Wo, bo)
    res = run_bass_kernel_spmd(nc, in_maps, list(range(NCORES)))
    acc = res.results[0]["out"].astype(np.float64)
    for c in range(1, NCORES):
        acc += res.results[c]["out"]
    # V bias folded host-side: concat rows carry +bv per head-dim, so the
    # device-side output is short exactly bv @ Wo^T (a constant row).
    acc += (bv.astype(np.float64) @ Wo.T.astype(np.float64))
    acc += bo.astype(np.float64)
    return acc.astype(np.float32)


# revision 26
# speedup vs baseline: 1.3003x; 1.1359x over previous
"""Trainium2 Bass kernel for nn_MultiHeadAttention (N=2048, D=1024, H=16, causal).

Sharding: 16 heads split across 8 NeuronCores (2 heads/core, tensor-parallel
on the head dim).  Each core projects Q^T/K^T (its 128 head-dims x full
sequence) and V for its heads, computes causal attention in scores-transposed
layout ([nk, nq] blocks, softmax along the nk partition axis), applies its
128-row slice of Wo, and writes an fp16 partial [2048, 1024] output.  The
host sums the 8 partials and adds bo + bv@Wo_slice^T ("all-reduce after W_o"
done host-side; the V bias is mathematically a constant output row, so it
never touches the device).

v2 redesign (75.8us -> target ~48us modeled):
  - PV flipped: probs blocks [128k, 128q] are the *stationary* operand and
    the ones-augmented V [128k, 65] the moving one, so each PV matmul costs
    65 free-columns instead of 128, and the softmax denominator lands on the
    *partition* axis of the PV output.  Normalisation collapses to a
    per-partition reciprocal + scale (no denominator staging row, no PE
    broadcast, no [64,512] reciprocal).
  - The normalized attention output [q, dims] is flipped back to the
    Wo-stationary layout [dims, q] with the DMA XBAR transpose (14ns per
    16x128 tile), not PE/DVE work.
  - Scores computed per 1024-wide q-supertile (T0/T1) so each causally
    trimmed [128, width<=1024] score block takes ONE exp instruction
    (52 exps total vs 80): the ~185ns fixed Act cost per instruction was
    ~15us of the baseline's Act-engine 46us.  T0 is emitted in 512-wide
    halves so the exp train starts as soon as the first q/k column group
    lands (~2.5us).
  - Input DMA order: all q/k fp8 column tiles first (the score pipeline's
    critical path), v f16 tiles deferred behind them; V projection, PV,
    normalize, transpose and Wo ride the emission-interleaved filler queue.
  - Wo staging copies (PSUM f32 -> SBUF f16) are spread DVE/Pool by slack.
"""
import os
import sys

for _p in ("/opt/trn_rl_repo", "/root/.axon_site/_ro/trn_rl_repo"):
    if os.path.isdir(_p) and _p not in sys.path:
        sys.path.append(_p)

import numpy as np

import concourse.bass as bass
import concourse.mybir as mybir
from concourse import bacc
from concourse.bass_utils import run_bass_kernel_spmd
from concourse.tile import TileContext
from contextlib import ExitStack

N = 2048
D = 1024
NCORES = 8
DL = 128

F32 = mybir.dt.float32
F16 = mybir.dt.float16
F8 = mybir.dt.float8e4

# fp8 Q/K path: q, k, Wq, Wk in e4m3 (weights host-scaled x16 to clear the
# e4m3 denormal floor; bq, bk scaled to match; the extra 16*16 factor on the
# scores folds into the exp scale).  V path, probs, attnT, Wo stay f16 --
# every attempted fp8 extension of those paths measured >2e-2 end-to-end.
EXP_SCALE = 0.125 / 256.0


def build_nc(opts=None):
    nc = bacc.Bacc("TRN2", target_bir_lowering=False, debug=False,
                   num_devices=NCORES)

    qP = nc.dram_tensor("qP", [128, 8, N], F8, kind="ExternalInput")
    kP = nc.dram_tensor("kP", [128, 8, N], F8, kind="ExternalInput")
    vP = nc.dram_tensor("vP", [128, 8, N], F16, kind="ExternalInput")
    # wq | wk | bqk-bits packed in one tensor: one front DMA instead of
    # three (each extra head-of-stream DMA costs ~600ns of queue overhead)
    wqkP = nc.dram_tensor("wqkP", [128, 8, 2 * DL + 8], F8,
                          kind="ExternalInput")
    wvP = nc.dram_tensor("wvP", [128, 8, DL], F16, kind="ExternalInput")
    woP = nc.dram_tensor("woP", [DL, D], F16, kind="ExternalInput")
    out = nc.dram_tensor("out", [N, D], F16, kind="ExternalOutput")

    AF = mybir.ActivationFunctionType
    ALU = mybir.AluOpType
    DR = mybir.MatmulPerfMode.DoubleRow

    with TileContext(nc) as tc, ExitStack() as ctx:
        const = ctx.enter_context(tc.tile_pool(name="const", bufs=1))
        big = ctx.enter_context(tc.tile_pool(name="big", bufs=1))
        colp = ctx.enter_context(tc.tile_pool(name="colp", bufs=1))
        probsp = ctx.enter_context(tc.tile_pool(name="probsp", bufs=1))
        rcqp = ctx.enter_context(tc.tile_pool(name="rcqp", bufs=2))
        outp = ctx.enter_context(tc.tile_pool(name="outp", bufs=3))

        # ---- input streaming: q tiles + weights on the SP queue, k tiles
        # on the Act HWDGE queue (descriptor generation overlaps the SP
        # queue's transfers, killing ~600ns/DMA of front serialization).
        # All q/k fp8 first (they feed the exp train, the kernel's pacer),
        # v f16 last (V-proj/PV/Wo ride the filler queue from ~20us on) ----
        wqk = const.tile([128, 8, 2 * DL + 8], F8)
        nc.sync.dma_start(wqk[:], wqkP[:])
        wq = wqk[:, :, 0:DL]
        wk = wqk[:, :, DL:2 * DL]
        bqk_t = wqk[:, 0, 2 * DL:2 * DL + 8].bitcast(F32)
        qc, kc, vc = [], [], []

        def load_col(lst, name, dram, c, dt, eng):
            t = colp.tile([128, 8, 512], dt, name=f"{name}{c}")
            eng.dma_start(t[:], dram[:, :, 512 * c:512 * (c + 1)])
            lst.append(t)

        for c in range(4):
            load_col(qc, "qc", qP, c, F8, nc.sync)
            load_col(kc, "kc", kP, c, F8, nc.scalar)
        wv = const.tile([128, 8, DL], F16)
        nc.sync.dma_start(wv[:], wvP[:])
        wo = const.tile([DL, D], F16)
        nc.sync.dma_start(wo[:], woP[:])
        for c in range(4):
            load_col(vc, "vc", vP, c, F16, nc.sync)

        ones64 = const.tile([1, 64], F16)
        nc.vector.memset(ones64[:], 1.0)
        ones512 = const.tile([1, 512], F16)
        nc.vector.memset(ones512[:], 1.0)
        # f16 identity for PE transposes: ones masked to the diagonal
        ident = const.tile([128, 128], F16)
        nc.vector.memset(ident[:], 1.0)
        nc.gpsimd.affine_select(
            out=ident[:], in_=ident[:], compare_op=mybir.AluOpType.is_ge,
            fill=0.0, base=0, pattern=[[1, 128]], channel_multiplier=-1)
        nc.gpsimd.affine_select(
            out=ident[:], in_=ident[:], compare_op=mybir.AluOpType.is_le,
            fill=0.0, base=0, pattern=[[1, 128]], channel_multiplier=-1)
        # dummy exp so the Act table load happens during the DMA front,
        # not in front of the first real score exp
        scratch = const.tile([1, 64], F16)
        nc.scalar.activation(scratch[:], ones64[:],
                             mybir.ActivationFunctionType.Exp, scale=1.0)

        # ---- persistent activations ----
        QT = big.tile([128, N], F16)
        KT = big.tile([128, N], F16)
        # Vaug[p_seq, head, seq_block, 65]: cols 0:64 projected V, col 64
        # ones (PV's moving operand; the ones column accumulates the softmax
        # denominator into PV-output column 64 for free).
        Vaug = big.tile([128, 2, 16, 65], F16)
        nc.vector.memset(Vaug[:, :, :, 64:65], 1.0)
        attnT = big.tile([128, N], F16)
        attnQ = [big.tile([128, 8, 128], F16, name=f"attnQ{T}")
                 for T in range(2)]

        # probs segments: (T, h, b) -> list of (qlo, qhi, pb_tile)
        probs_seg = {}

        # PSUM budget (8 banks): scp 2x[128,2,512] (4) for scores/proj/
        # vproj, wps 1x[128,2,512] (2) for Wo matmuls, pvqp 2x[128,4,65]
        # (2) for PV accumulators + transpose staging
        with tc.tile_pool(name="scp", bufs=2, space="PSUM") as scp, \
             tc.tile_pool(name="wps", bufs=1, space="PSUM") as wps, \
             tc.tile_pool(name="pvqp", bufs=2, space="PSUM") as pvqp:

            # The PE SEQ is in-order and sem-waits block it, so emission
            # interleaves independent "filler" work (V projection, PV
            # strip sweeps, normalize, transpose, Wo) between score blocks,
            # paced against the Activation-engine exp train.
            fillers = []

            def emit_score(T, h, b, qlo, qhi):
                # one causally-trimmed score block: k in [128b, 128b+128),
                # q in [qlo, qhi).  Matmuls split at PSUM bank boundaries;
                # ONE exp covers the whole [128, qhi-qlo] region.
                w = qhi - qlo
                r0, r1 = 64 * h, 64 * (h + 1)
                sc = scp.tile([128, 2, 512], F32, name="sc")
                flat = sc.rearrange("p a b -> p (a b)")
                for c0 in range(0, w, 512):
                    c1 = min(c0 + 512, w)
                    nc.tensor.matmul(
                        flat[:, c0:c1],
                        KT[r0:r1, 128 * b:128 * (b + 1)],
                        QT[r0:r1, qlo + c0:qlo + c1],
                        start=True, stop=True, tile_position=(64 * h, 0))
                pb = probsp.tile([128, w], F16, name=f"pb{T}{h}{b}_{qlo}")
                nc.scalar.activation(pb[:], flat[:, 0:w], AF.Exp,
                                     scale=EXP_SCALE)
                if 128 * b >= qlo:
                    # diagonal chunk starts this segment: causal-mask it
                    nc.gpsimd.affine_select(
                        out=pb[:, 0:128], in_=pb[:, 0:128],
                        compare_op=ALU.is_ge, fill=0.0,
                        base=0, pattern=[[1, 128]], channel_multiplier=-1)
                probs_seg.setdefault((T, h, b), []).append((qlo, qhi, pb))

            def probs_slice(T, h, b, s):
                q0 = 1024 * T + 128 * s
                for qlo, qhi, pb in probs_seg[(T, h, b)]:
                    if qlo <= q0 < qhi:
                        return pb[:, q0 - qlo:q0 - qlo + 128]
                raise AssertionError((T, h, b, s))

            def emit_vproj(c, j):
                # seq block blk = 4c+j -> vp region [:, 128j:128j+128]
                blk = 4 * c + j
                if j == 0:
                    emit_vproj.vp = scp.tile([128, 2, 512], F32, name="sc")
                vp = emit_vproj.vp.rearrange("p a b -> p (a b)")
                for u in range(8):
                    nc.tensor.matmul(
                        vp[:, 128 * j:128 * (j + 1)],
                        vc[c][:, u, 128 * j:128 * (j + 1)],
                        wv[:, u, :], start=(u == 0), stop=(u == 7))

            def emit_vcopy(c):
                vp = emit_vproj.vp.rearrange(
                    "p a (bb g d) -> p a g bb d", bb=4, g=2, d=64)
                nc.vector.tensor_scalar_mul(
                    Vaug[:, 0:2, 4 * c:4 * (c + 1), 0:64],
                    vp[:, 0, :, :, :], 1.0)

            pvq_ref = {}

            def emit_pv(T, h, s):
                # strip s of supertile T: accumulate over all k blocks
                # b = 0..8T+s into pvq[128q, 65] (probs stationary, Vaug
                # moving: 65 free-cols per matmul; col 64 = denominator).
                sg, si = s // 4, s % 4
                if si == 0:
                    pvq_ref[(T, h, sg)] = pvqp.tile([128, 4, 65], F32,
                                                    name="pvq")
                pvq = pvq_ref[(T, h, sg)]
                last = 8 * T + s
                for b in range(last + 1):
                    nc.tensor.matmul(
                        pvq[:, si, 0:65],
                        probs_slice(T, h, b, s),
                        Vaug[:, h, b, 0:65],
                        start=(b == 0), stop=(b == last))

            def emit_norm(T, h, sg):
                pvq = pvq_ref[(T, h, sg)]
                rcq = rcqp.tile([128, 4, 1], F32, name="rcq")
                nc.vector.reciprocal(rcq[:], pvq[:, :, 64:65])
                nc.vector.tensor_mul(
                    attnQ[T][:, 4 * sg:4 * (sg + 1), 64 * h:64 * (h + 1)],
                    pvq[:, :, 0:64],
                    rcq.broadcast_to((128, 4, 64)))

            def emit_norm_strip(T, h, s):
                # per-strip normalize for the final strip-group, so the
                # tail pipelines strip-by-strip instead of per-quad
                sg, si = s // 4, s % 4
                pvq = pvq_ref[(T, h, sg)]
                rcq = rcqp.tile([128, 4, 1], F32, name="rcq")
                nc.vector.reciprocal(rcq[:, 0:1, :], pvq[:, si:si + 1, 64:65])
                nc.vector.tensor_mul(
                    attnQ[T][:, s:s + 1, 64 * h:64 * (h + 1)],
                    pvq[:, si:si + 1, 0:64],
                    rcq[:, 0:1, :].broadcast_to((128, 1, 64)))

            def emit_transpose(T, s, final=False):
                # PE transpose (53ns) + DVE copy: much shorter latency than
                # the DMA XBAR path and keeps SP/HWDGE free for output
                # DMAs.  PSUM slot from the pvq pool (freed by the norm
                # just before this), NOT wps: wps slots recycle through the
                # slow Wo/staging chain and would stall the PE stream here.
                # The final per-strip chain uses the by-then-idle score
                # pool instead (the h1 pvq quad stays live across all four
                # of its strip-norms, so its pool can't rotate there).
                m = 8 * T + s
                pool = scp if final else pvqp
                tp = pool.tile([128, 128], F16,
                               name="sc" if final else "pvq")
                nc.tensor.transpose(tp[:], attnQ[T][:, s, :], ident[:])
                nc.vector.tensor_scalar_mul(
                    attnT[:, 128 * m:128 * (m + 1)], tp[:], 1.0)

            def emit_wo(T, s, stage):
                # both [128,512] halves into one rotating 2-bank PSUM pair,
                # ONE f32->f16 staging op, one output DMA per 128-row band
                m = 8 * T + s
                ob = outp.tile([128, 1024], F16, name="ob")
                wp = wps.tile([128, 2, 512], F32, name="wp")
                for u in range(2):
                    nc.tensor.matmul(wp[:, u, :],
                                     attnT[:, 128 * m:128 * (m + 1)],
                                     wo[:, 512 * u:512 * (u + 1)],
                                     start=True, stop=True)
                src = wp.rearrange("p a b -> p (a b)")
                if stage == "act":
                    nc.scalar.activation(ob[:], src[:], AF.Copy)
                elif stage == "pool":
                    nc.gpsimd.tensor_scalar_mul(ob[:], src[:], 1.0)
                else:
                    nc.vector.tensor_scalar_mul(ob[:], src[:], 1.0)
                nc.sync.dma_start(out[128 * m:128 * (m + 1), :], ob[:])

            proj_ps = {}

            def emit_proj(g, lo=0, hi=512):
                # QT/KT columns [512g+lo : 512g+hi] from qc[g]/kc[g], fp8
                # DoubleRow (256-wide contraction per pass); group 0 comes
                # in 256-col halves chasing the front DMA
                if lo == 0:
                    proj_ps[g] = scp.tile([128, 2, 512], F32, name="sc")
                ps = proj_ps[g]
                for src_c, w, half in ((qc[g], wq, 0), (kc[g], wk, 1)):
                    for jj in range(4):
                        nc.tensor.matmul(
                            ps[:, half, lo:hi], w[:, 2 * jj:2 * jj + 2, :],
                            src_c[:, 2 * jj:2 * jj + 2, lo:hi],
                            start=(jj == 0), stop=(jj == 3),
                            perf_mode=DR)
                nc.vector.tensor_scalar_add(
                    QT[:, 512 * g + lo:512 * g + hi], ps[:, 0, lo:hi],
                    bqk_t[:, 0:1])
                nc.vector.tensor_scalar_add(
                    KT[:, 512 * g + lo:512 * g + hi], ps[:, 1, lo:hi],
                    bqk_t[:, 1:2])

            def warmup(n=1):
                # keep the tensor engine busy through the DMA front so the
                # p-state ramp completes before the first projection
                for _ in range(n):
                    wu = wps.tile([128, 2, 512], F32, name="wp")
                    nc.tensor.matmul(wu[0:64, 0, :], ones64[:], ones512[:],
                                     start=True, stop=True)

            import math

            # Two filler queues: "fast" units are dep-shallow (V-proj, PV
            # sweeps, norms, transposes) and cheap; "slow" units are the Wo
            # chains whose staging recycles the single wps PSUM pair.  Slow
            # units pop at most one per score pair so their ~1.5us chains
            # never dam the in-order PE stream (which would starve the exp
            # train - the kernel's pacer).
            slow = []
            pairs_left = [32]

            def pop_fillers():
                if slow:
                    slow.pop(0)()
                k = max(2, min(5, math.ceil(
                    len(fillers) / max(1, pairs_left[0]))))
                for _ in range(k):
                    if fillers:
                        fillers.pop(0)()

            def queue_vproj(c):
                fillers.extend(lambda c=c, j=j: emit_vproj(c, j)
                               for j in range(4))
                fillers.append(lambda c=c: emit_vcopy(c))

            def queue_pv(T, h, sg):
                for si in range(4):
                    fillers.append(lambda T=T, h=h, s=4 * sg + si:
                                   emit_pv(T, h, s))
                fillers.append(lambda T=T, h=h, sg=sg:
                               emit_norm(T, h, sg))

            def queue_tail(T, sg):
                for si in range(4):
                    s = 4 * sg + si
                    fillers.append(lambda T=T, s=s: emit_transpose(T, s))
                    slow.append(lambda T=T, s=s: emit_wo(T, s, "pool"))

            # ---------------- emission schedule ----------------
            # Heads are interleaved within every score row so both heads'
            # probs for a k-block land together and the per-strip-group
            # backend (PV/norm/transpose/Wo) unblocks mid-row.  A popped
            # filler executes ~2 score pairs behind the exp frontier (PSUM
            # slot rotation), so queue points only need their dependencies
            # EMITTED; the v-tile DMAs (~17-27us) land comfortably ahead
            # of their consumers' execution times.
            warmup(4)
            emit_proj(0)
            warmup(1)

            # T0: 512-wide A pass (needs only qc0/kc0) starts the exp
            # train ~7us in; 1024-wide B pass after proj(1)
            for b in range(4):
                for h in range(2):
                    emit_score(0, h, b, 128 * b, 512)
                if b == 0:
                    emit_proj(1)
            for b in range(8):
                for h in range(2):
                    emit_score(0, h, b, max(512, 128 * b), 1024)
                # proj(2)/proj(3) mid-B: their scp slots must be acquired
                # well before T1's first score tile or T1's exp start slips
                # a full PSUM-rotation behind T0's exp tail
                if b == 2:
                    emit_proj(2)
                if b == 4:
                    emit_proj(3)
            # T0's backend: all pb deps emitted; Vaug deps (vc0/vc1) land
            # well before any of this executes
            queue_vproj(0)
            queue_pv(0, 0, 0)
            queue_pv(0, 1, 0)
            queue_tail(0, 0)
            queue_vproj(1)
            queue_pv(0, 0, 1)
            queue_pv(0, 1, 1)
            queue_tail(0, 1)
            queue_vproj(2)
            queue_vproj(3)

            for b in range(16):
                for h in range(2):
                    emit_score(1, h, b, max(1024, 128 * b), 2048)
                    pairs_left[0] -= 1
                    pop_fillers()
                if b == 11:
                    # pb(T1, *, b<=11) emitted: strip-group 0 (strips 0..3
                    # sweep k-blocks 0..11) ready for both heads
                    queue_pv(1, 0, 0)
                    queue_pv(1, 1, 0)
                    queue_tail(1, 0)
            # final strip-group (T1, sg1): h0 as a quad, then per-strip
            # pipelining for h1 so the post-exp tail is one strip deep;
            # staging on DVE/Act, both idle once the exp train ends
            queue_pv(1, 0, 1)
            for si in range(4):
                s = 4 + si
                fillers.append(lambda s=s: emit_pv(1, 1, s))
                fillers.append(lambda s=s: emit_norm_strip(1, 1, s))
                fillers.append(lambda s=s: emit_transpose(1, s, final=True))
                st = ("pool", "dve", "act", "dve")[si]
                fillers.append(lambda s=s, st=st: emit_wo(1, s, st))
            # drain: fast queue first (it holds the producers - transposes -
            # of any remaining slow Wo units), then the slow leftovers
            while fillers:
                fillers.pop(0)()
            while slow:
                slow.pop(0)()

    nc.compile()
    return nc


def make_in_maps(q, k, v, Wq, bq, Wk, bk, Wv, bv, Wo, bo):
    import ml_dtypes
    fp8 = ml_dtypes.float8_e4m3
    f32 = np.float32
    WSCALE = 16.0

    def pack_cols(x, dt):
        # [N, D] input -> x.T [D, N] -> [128, 8, N] with row (j*128+p) at
        # [p, j, :]
        xt = np.ascontiguousarray(x.T.astype(f32))
        return np.ascontiguousarray(
            xt.reshape(8, 128, N).transpose(1, 0, 2)).astype(dt)

    qPa, kPa = pack_cols(q, fp8), pack_cols(k, fp8)
    vPa = pack_cols(v, np.float16)
    WqT = Wq.T.astype(f32) * WSCALE
    WkT = Wk.T.astype(f32) * WSCALE
    WvT = Wv.T.astype(f32)
    WoT = Wo.T.astype(f32)

    def pack_w(WT, c, dt):
        # [D, DL] column slice -> [128, 8, DL]
        sl = np.ascontiguousarray(WT[:, DL * c:DL * (c + 1)])
        return np.ascontiguousarray(
            sl.reshape(8, 128, DL).transpose(1, 0, 2)).astype(dt)

    in_maps = []
    for c in range(NCORES):
        d0 = DL * c
        # wq | wk | bqk(f32, bit-packed into 8 fp8 lanes of j=0)
        wqk = np.zeros((128, 8, 2 * DL + 8), dtype=np.uint8)
        wqk[:, :, 0:DL] = pack_w(WqT, c, fp8).view(np.uint8)
        wqk[:, :, DL:2 * DL] = pack_w(WkT, c, fp8).view(np.uint8)
        bqk = np.ascontiguousarray(
            np.stack([bq[d0:d0 + DL] * WSCALE,
                      bk[d0:d0 + DL] * WSCALE], axis=1)).astype(f32)
        wqk[:, 0, 2 * DL:2 * DL + 8] = bqk.view(np.uint8).reshape(128, 8)
        in_maps.append({
            "qP": qPa, "kP": kPa, "vP": vPa,
            "wqkP": wqk.view(fp8),
            "wvP": pack_w(WvT, c, np.float16),
            "woP": np.ascontiguousarray(WoT[d0:d0 + DL, :]).astype(np.float16),
        })
    return in_maps


_NC_CACHE = None


def _get_nc():
    global _NC_CACHE
    if _NC_CACHE is None:
        _NC_CACHE = build_nc()
    return _NC_CACHE


def kernel(q, k, v, Wq, bq, Wk, bk, Wv, bv, Wo, bo):
    """Full-input / full-output entry point (harness contract)."""
    q, k, v = np.asarray(q), np.asarray(k), np.asarray(v)
    Wq, bq, Wk, bk = np.asarray(Wq), np.asarray(bq), np.asarray(Wk), np.asarray(bk)
    Wv, bv, Wo, bo = np.asarray(Wv), np.asarray(bv), np.asarray(Wo), np.asarray(bo)
    nc = _get_nc()
    in_maps = make_in_maps(q, k, v, Wq, bq, Wk, bk, Wv, bv, Wo, bo)
    res = run_bass_kernel_spmd(nc, in_maps, list(range(NCORES)))
    acc = res.results[0]["out"].astype(np.float64)
    for c in range(1, NCORES):
        acc += res.results[c]["out"]
    # V bias folded host-side: concat rows carry +bv per head-dim, so the
    # device-side output is short exactly bv @ Wo^T (a constant row).
    acc += (bv.astype(np.float64) @ Wo.T.astype(np.float64))
    acc += bo.astype(np.float64)
    return acc.astype(np.float32)
